# revision 36
# baseline (speedup 1.0000x reference)
"""Trainium2 Bass kernel for 3D multi-head attention (nn_Attention3D).

Problem: x [1, 16, 16, 16, 528] -> full attention over N=4096 tokens,
8 heads of dim 66, qkv + out projections.

Sharding: one head per NeuronCore (8 cores). Each core computes its
head's q/k/v projections, full 4096x4096 attention, and its partial
contribution to the output projection. Host sums the 8 partials and
adds the output bias.

Key layout decisions (all host-side prep, free):
  - x is pre-transposed on host to xT [640, 4096] (C on partitions),
    with row 528 = 1.0 (bias row) and rows 529-639 = 0 padding; qkv
    biases fold into the weight matmuls via the ones row.
  - Scores are computed transposed ([k-tokens, q-tokens]) so softmax's
    sum runs over the partition dim via a ones-column in the v weights
    (the attention-value matmul accumulates the denominator for free),
    and no transposes are ever needed.
  - Scores are produced directly in "exp2-bits" units: hd^-0.5 * log2e
    * 2^23 is folded into wq/bq, and a global -0.5*2^23 shift rides on
    the spare qT/kT row 66 (a constant shift of all scores cancels in
    softmax normalization).  This lets the exp over the 16.7M-score
    matrix be SPLIT across two engines:
      * ScalarE: native Exp activation with scale=ln2*2^-23,
        bias=+0.5*ln2 in the free affine -> bf16 E.
      * VectorE: a one-instruction custom DVE op that assembles the
        fp32 BIT PATTERN of 2^t arithmetically (magic-add floor range
        reduction, quadratic mantissa polynomial, +(127+a0)*2^23) and
        writes through an int32-convert output.  The attention-value
        matmul then reads the high 2 bytes of each fp32 via a stride-2
        bf16 bitcast AP (truncation bias cancels in the softmax
        normalization since the denominator sees the same values).
    Baseline had exp 100% on ScalarE (140us, co-critical with the PE);
    the split makes phase B purely PE-bound.
  - All matmuls are bf16 (1 cyc/row + fast weight load), including the
    output projection (baseline used float32r there: ~660ns/MM vs
    ~213ns bf16).  PSUM accumulation is always fp32.
  - Phase B runs a cross-block software pipeline: the AV matmuls trail
    the score/exp stream by 3 two-chunk groups (even across q-block
    boundaries), 3x2 score PSUM banks + 2 oT-accumulator banks.  The
    last two groups of each block exp entirely on ScalarE so VectorE
    is free for the block-end reciprocal + normalize.
  - The softmax normalization is folded into the oT copy as a
    column-wise tensor_tensor multiply against the gpsimd
    partition-broadcast reciprocal, so the output projection needs no
    per-token scale.  That lets phase C emit y TRANSPOSED
    ([embed, token]): one weight load per 128-wide embed chunk, N=512
    matmuls, and one ~0.5MB DMA per embed-chunk half with
    8KB-contiguous DRAM runs (token-major y was stuck at ~65GB/s per
    queue on 1KB runs).  y partials are bf16, summed in fp32 on host.

Measured: 251us (baseline) -> ~184us/core at full clock (~213us if the
chip is in the P0 2.0GHz power state), rel err ~2.3e-3.
"""

import numpy as np

import ml_dtypes

BF16_NP = ml_dtypes.bfloat16

EMBED = 528
HD = 66
NHEADS = 8
NT = 4096
NCH = 5  # contraction chunks of 128 (640 = 528 + bias row + pad)

# exp2-bits constants (see exp_dve.py experiment)
MAGIC = 1.5 * 2.0**46
A0 = 0.414839277933763
A1 = 0.9948016962806719
A2 = 0.33717699739561857
BIAS_E = (127.0 + A0) * 2.0**23
LN2 = float(np.log(2.0))
LN2_SC = LN2 / 2.0**23
C_SCORE = float(HD**-0.5 / LN2 * 2.0**23)  # fold into wq/bq
SHIFT_K = -0.5 * 2.0**23  # qT/kT row 66 rank-1 constant shift


def _register_exp2():
    """Register the one-instruction DVE 2^x-bits op (idempotent)."""
    import concourse.dve_ops as dve_ops

    for op in dve_ops.OPS:
        if op.name == "EXP2_BITS_ANT":
            return op

    from concourse.dve_spec import (
        Spec,
        Src0,
        C0,
        C1,
        C2,
        C3,
        lower,
        _spill_c3_to_src1,
    )
    from concourse.dve_uop import DveOpSpec

    # Input is ts23 = (t - 0.5)*2^23; out_bits = fp32 bits of 2^t:
    #   p   = ts23 + 1.5*2^46  (rounds ts23 to nearest multiple of 2^23)
    #   i23 = p - 1.5*2^46     (= floor(t)*2^23 exact)
    #   X   = ts23 - i23       (= (f-0.5)*2^23, f = t-floor(t), exact)
    #   m   = X*(a1 + (a2/2^23)*X)
    #   y   = m + i23 + (127 + a0)*2^23
    # with a0 + a1 x + a2 x^2 ~= sqrt2*2^x - 1 = 2^f - 1 >= 0 on x in
    # [-0.5, 0.5]; int32-convert writeback turns the value into bits.
    p = Src0 + C0
    i23 = p - C0
    x = Src0 - i23
    m = (x * C3 + C1) * x
    y = (m + i23) + C2

    def _ref(in0, in1, c0, c1, c2):
        f32 = np.float32
        t = in0.astype(f32)
        pp = (t + f32(c0)).astype(f32)
        ii = (pp - f32(c0)).astype(f32)
        xx = (t - ii).astype(f32)
        mm = (xx * ((in1 * xx).astype(f32) + f32(c1))).astype(f32)
        return (mm + ii + f32(c2)).astype(f32)

    spec = Spec(body=_spill_c3_to_src1(y), reference=_ref)
    shas = {}
    for ver in ("v3", "v4"):
        try:
            s = DveOpSpec(
                name="EXP2_BITS_ANT", opcode=None, uops=lower(spec, ver=ver),
                rd1_en=True,
            )
            shas[ver] = s.sha(ver)
        except Exception:
            pass
    op = dve_ops.DveOp("EXP2_BITS_ANT", spec, subdim=False, uops_sha=shas)
    dve_ops.OPS.append(op)
    dve_ops._SUB_OPCODE_FOR_NAME[op.name] = (
        dve_ops._CUSTOM_DVE_ROW_BASE + len(dve_ops.OPS) - 1
    )
    dve_ops.CUSTOM_DVE_SPECS[op.name] = op.spec
    return op


def _build_nc(nt=NT):
    import concourse.tile as tile
    from concourse import bacc, mybir

    exp2_op = _register_exp2()

    F32 = mybir.dt.float32
    I32 = mybir.dt.int32
    BF16 = mybir.dt.bfloat16
    AF = mybir.ActivationFunctionType

    nkc = nt // 128  # k-token chunks
    nqb = nt // 512  # q-token blocks

    nc = bacc.Bacc("TRN2", target_bir_lowering=False, debug=False)
    xT_d = nc.dram_tensor("xT", [128, NCH, nt], BF16, kind="ExternalInput").ap()
    wq_d = nc.dram_tensor("wq", [128, NCH, 128], BF16, kind="ExternalInput").ap()
    wk_d = nc.dram_tensor("wk", [128, NCH, 128], BF16, kind="ExternalInput").ap()
    wv_d = nc.dram_tensor("wv", [128, NCH, HD + 2], BF16, kind="ExternalInput").ap()
    wp_d = nc.dram_tensor("wp", [128, EMBED], BF16, kind="ExternalInput").ap()
    cs_d = nc.dram_tensor("cs", [128, 2], F32, kind="ExternalInput").ap()
    y_d = nc.dram_tensor("y", [EMBED, nt], BF16, kind="ExternalOutput").ap()

    with tile.TileContext(nc) as tc:
        with (
            tc.tile_pool(name="const", bufs=1) as constp,
            tc.tile_pool(name="persist", bufs=1) as pp,
        ):
            wq = constp.tile([128, NCH, 128], BF16, name="wq_sb")
            wk = constp.tile([128, NCH, 128], BF16, name="wk_sb")
            wv = constp.tile([128, NCH, HD + 2], BF16, name="wv_sb")
            wp = constp.tile([128, EMBED], BF16, name="wp_sb")
            cs = constp.tile([128, 2], F32, name="cs_sb")
            warm = constp.tile([128, 1], F32, name="warm_sb")
            nc.sync.dma_start(cs[:], cs_d[:])
            c3_ap = cs[:, 0:1]
            cb_ap = cs[:, 1:2]

            # preload the exp table set early (2.7us, overlaps phase A)
            nc.scalar.activation(warm[:], cs[:, 0:1], AF.Exp, scale=0.0)

            qT = pp.tile([128, nt], BF16, name="qT")
            kT = pp.tile([128, nt], BF16, name="kT")
            oT = pp.tile([128, nt], BF16, name="oT")
            vaug = pp.tile([128, nkc, HD + 2], BF16, name="vaug")
            # junk rows 68-127 of oT feed the projection lhsT; NaN*0 = NaN.
            # (partition bases must be 32-aligned, so clear all of oT; the
            # live rows 0-67 are overwritten per q-block in phase B)
            nc.gpsimd.memset(oT[:, :], 0.0)
            # first partition_broadcast pays a ~7us gpsimd ucode load;
            # warm it up here where gpsimd is idle
            nc.gpsimd.partition_broadcast(warm[:], cs[0:1, 0:1])

            # ---------------- Phase A: qkv projections ----------------
            with (
                tc.tile_pool(name="xp", bufs=1) as xp,
                tc.tile_pool(name="psA", bufs=4, space="PSUM") as psA,
            ):
                xT = xp.tile([128, NCH, nt], BF16, name="xT_sb")
                # two HW DMA queues (sync + scalar), ~600ns trigger cost
                # each: block 0 lands chunk-by-chunk on sync so the first
                # matmul starts ~9us in; everything else is block-sized.
                b0 = slice(0, 512)
                for c in range(NCH):
                    nc.sync.dma_start(xT[:, c, b0], xT_d[:, c, b0])
                nc.scalar.dma_start(wq[:], wq_d[:])
                nc.scalar.dma_start(wk[:], wk_d[:])
                for b in range(1, nqb):
                    qs = slice(b * 512, (b + 1) * 512)
                    eng = nc.scalar if b % 2 else nc.sync
                    eng.dma_start(xT[:, :, qs], xT_d[:, :, qs])
                nc.scalar.dma_start(wv[:], wv_d[:])
                nc.sync.dma_start(wp[:], wp_d[:])

                # interleave the q and k accumulation chains (independent
                # PSUM banks) so consecutive PE matmuls pipeline
                for b in range(nqb):
                    qs = slice(b * 512, (b + 1) * 512)
                    ps_q = psA.tile([128, 512], F32, tag="qk", name="ps_q")
                    ps_k = psA.tile([128, 512], F32, tag="qk", name="ps_k")
                    for c in range(NCH - 1):
                        for w, ps in ((wq, ps_q), (wk, ps_k)):
                            nc.tensor.matmul(
                                ps[:],
                                w[:, c, :],
                                xT[:, c, qs],
                                start=(c == 0),
                                stop=False,
                                skip_group_check=True,
                            )
                    # chunk 4 has only 17 live rows (features 512-527 +
                    # bias row); run q's as a K=32 row-tile on rows 0-31
                    # and k's on rows 32-63 (host replicates the rows
                    # there) so the two matmuls stream concurrently
                    nc.tensor.matmul(
                        ps_q[:],
                        wq[0:32, NCH - 1, :],
                        xT[0:32, NCH - 1, qs],
                        start=False,
                        stop=True,
                        skip_group_check=True,
                    )
                    nc.tensor.matmul(
                        ps_k[:],
                        wk[32:64, NCH - 1, :],
                        xT[32:64, NCH - 1, qs],
                        start=False,
                        stop=True,
                        skip_group_check=True,
                        tile_position=(32, 0),
                    )
                    nc.vector.tensor_copy(qT[:, qs], ps_q[:])
                    nc.vector.tensor_copy(kT[:, qs], ps_k[:])
                # v: two token-block chains in flight; copies on ScalarE
                # (VectorE owns the qT/kT casts, ScalarE is idle here)
                for t0 in range(0, nkc, 2):
                    psvs = [
                        psA.tile([128, HD + 2], F32, tag="v", name="ps_v")
                        for _ in range(2)
                    ]
                    for c in range(NCH):
                        for i in range(2):
                            ts_ = slice((t0 + i) * 128, (t0 + i + 1) * 128)
                            nc.tensor.matmul(
                                psvs[i][:],
                                xT[:, c, ts_],
                                wv[:, c, :],
                                start=(c == 0),
                                stop=(c == NCH - 1),
                            )
                    for i in range(2):
                        nc.scalar.activation(
                            vaug[:, t0 + i, :], psvs[i][:], AF.Copy
                        )

            # ---------------- Phase B: attention ----------------
            # 2-chunk score groups, triple-buffered (3x2 PSUM banks) + 2
            # banks for the oT accumulator = 8.  Per group, chunk 0's exp
            # runs on ScalarE (bf16 E) and chunk 1's on VectorE (exp2-bits
            # int32) -- different PSUM banks, so the reads are parallel.
            # AV(g) is emitted after scores(g+2), giving the exps ~1.3us
            # of PE-time cover.  The softmax normalization is folded into
            # the oT copy (column-wise multiply against the partition-
            # broadcast reciprocal), so phase C needs no per-token scale
            # and can emit y transposed with large DMA runs.
            with (
                tc.tile_pool(name="eps", bufs=4) as eps,
                tc.tile_pool(name="edp", bufs=4) as edp,
                tc.tile_pool(name="rp", bufs=2) as rp,
                tc.tile_pool(name="rbp", bufs=2) as rbp,
                tc.tile_pool(name="psS", bufs=3, space="PSUM") as psS,
                tc.tile_pool(name="psO", bufs=2, space="PSUM") as psO,
            ):
                def emit_av(b, o_ps, g0, Es, Ed):
                    qs = slice(b * 512, (b + 1) * 512)
                    for j in range(2):
                        kc = g0 + j
                        if Ed is None:
                            rhs = Es[:, j, :]
                        elif j == 0:
                            rhs = Es[:]
                        else:
                            rhs = (
                                Ed[:]
                                .bitcast(BF16)
                                .rearrange("p (n two) -> p n two", two=2)[:, :, 1]
                            )
                        nc.tensor.matmul(
                            o_ps[:],
                            vaug[:, kc, :],
                            rhs,
                            start=(kc == 0),
                            stop=(kc == nkc - 1),
                            skip_group_check=True,
                        )
                    if g0 + 2 == nkc:
                        # block finished: defer the normalize a few groups
                        # so the recip+mul don't sit in VectorE's queue
                        # ahead of the next block's exps at the boundary
                        norm_q.append((b, o_ps))

                def emit_norm():
                    nb, no_ps = norm_q.pop(0)
                    nqs = slice(nb * 512, (nb + 1) * 512)
                    recip = rp.tile([1, 512], F32, name="recip")
                    nc.vector.reciprocal_approx_fast(recip[:], no_ps[0:1, :])
                    recipB = rbp.tile([128, 512], F32, name="recipB")
                    nc.gpsimd.partition_broadcast(recipB[:], recip[:])
                    nc.vector.tensor_mul(
                        oT[: HD + 2, nqs], no_ps[:], recipB[: HD + 2, :]
                    )

                # the AV pipeline trails the score/exp stream by 3 groups
                # ACROSS block boundaries, so the last exps of a block get
                # PE cover from the next block's first scores
                pend = []
                norm_q = []
                for b in range(nqb):
                    qs = slice(b * 512, (b + 1) * 512)
                    o_ps = psO.tile([HD + 2, 512], F32, name="o_ps")
                    for g in range(nkc // 2):
                        g0 = 2 * g
                        sc = psS.tile([128, 2, 512], F32, tag="sc", name="sc")
                        for j in range(2):
                            kc = g0 + j
                            nc.tensor.matmul(
                                sc[:, j, :],
                                kT[:, kc * 128 : (kc + 1) * 128],
                                qT[:, qs],
                                start=True,
                                stop=True,
                            )
                        if g >= nkc // 2 - 2:
                            # last two groups: both chunks on ScalarE (one
                            # wide activation) so VectorE is free for the
                            # block-end recip + normalize without stalling
                            # the next block's first AVs
                            Es = eps.tile(
                                [128, 2, 512], BF16, tag="Es2", name="Es2"
                            )
                            nc.scalar.activation(
                                Es[:],
                                sc[:],
                                AF.Exp,
                                scale=LN2_SC,
                                bias=cb_ap,
                            )
                            Ed = None
                        else:
                            Es = eps.tile([128, 512], BF16, tag="Es", name="Es")
                            Ed = edp.tile([128, 512], I32, tag="Ed", name="Ed")
                            nc.scalar.activation(
                                Es[:], sc[:, 0, :], AF.Exp, scale=LN2_SC, bias=cb_ap
                            )
                            nc.vector._custom_dve(
                                exp2_op,
                                out=Ed[:],
                                in0=sc[:, 1, :],
                                in1=c3_ap,
                                s0=MAGIC,
                                s1=A1,
                                imm2=BIAS_E,
                            )
                        pend.append((b, o_ps, g0, Es, Ed))
                        if len(pend) > 3:
                            emit_av(*pend.pop(0))
                        if norm_q and g == 4:
                            emit_norm()
                for p_ in pend:
                    emit_av(*p_)
                while norm_q:
                    emit_norm()

            # ---------------- Phase C: output projection ----------------
            # y is produced TRANSPOSED ([embed, token]): one weight load
            # per 128-wide embed chunk, N=512 matmuls over token blocks,
            # single-bank PSUM tiles, and one ~1MB DMA per embed chunk
            # with 8KB-contiguous runs (token-major y needed 32 DMAs of
            # 1056B runs at ~65GB/s/queue, which dominated phase C).
            with (
                tc.tile_pool(name="yp", bufs=2) as yp,
                tc.tile_pool(name="psY", bufs=4, space="PSUM") as psY,
            ):
                for e in range(5):
                    ew = 128 if e < 4 else EMBED - 512  # last chunk: 16
                    es_ = slice(e * 128, e * 128 + ew)
                    ysb = yp.tile([128, nt], BF16, tag="ysb", name="ysb")
                    for th in range(nqb // 2):
                        yps = psY.tile([128, 2, 512], F32, name="yps")
                        for i in range(2):
                            tb = 2 * th + i
                            ts_ = slice(tb * 512, (tb + 1) * 512)
                            nc.tensor.matmul(
                                yps[:ew, i, :],
                                wp[:, es_],
                                oT[:, ts_],
                                start=True,
                                stop=True,
                            )
                        hs = slice(2 * th * 512, (2 * th + 2) * 512)
                        if th % 2 == 0:
                            nc.vector.tensor_copy(ysb[:ew, hs], yps[:ew, :, :])
                        else:
                            nc.scalar.activation(
                                ysb[:ew, hs], yps[:ew, :, :], AF.Copy
                            )
                    h = nt // 2
                    engs = (nc.sync, nc.scalar) if e % 2 == 0 else (nc.scalar, nc.sync)
                    engs[0].dma_start(y_d[es_, :h], ysb[:ew, :h])
                    engs[1].dma_start(y_d[es_, h:], ysb[:ew, h:])

    nc.compile()
    return nc


def _prep_inputs(x, w_qkv, b_qkv, w_proj, nt):
    """Host-side shard prep: returns list of 8 in_maps."""
    x = np.asarray(x, dtype=np.float32)
    w_qkv = np.asarray(w_qkv, dtype=np.float32)
    b_qkv = np.asarray(b_qkv, dtype=np.float32)
    w_proj = np.asarray(w_proj, dtype=np.float32)

    xt = x.reshape(nt, EMBED)
    xT_pad = np.zeros((NCH * 128, nt), dtype=np.float32)
    xT_pad[:EMBED] = xt.T
    xT_pad[EMBED] = 1.0
    # replica of chunk-4's 17 live rows at partition offset 32 for the
    # k-projection row-tile (rows 544-560 = rows 512-528)
    xT_pad[544:561] = xT_pad[512:529]
    # [128, NCH, nt]: partition-major so one DMA covers a token block
    xT_in = np.ascontiguousarray(
        xT_pad.reshape(NCH, 128, nt).transpose(1, 0, 2)
    ).astype(BF16_NP)

    cs = np.zeros((128, 2), dtype=np.float32)
    cs[:, 0] = A2 / 2.0**23
    cs[:, 1] = 0.5 * LN2

    in_maps = []
    for h in range(NHEADS):
        sl_q = slice(h * HD, (h + 1) * HD)
        sl_k = slice(EMBED + h * HD, EMBED + (h + 1) * HD)
        sl_v = slice(2 * EMBED + h * HD, 2 * EMBED + (h + 1) * HD)

        # q side carries the hd^-0.5 * log2e * 2^23 score scaling; the
        # spare column 66 carries the global -0.5*2^23 shift (qT66 = 1,
        # kT66 = -2^22) that the exp2-bits op's range reduction needs.
        wq_t = np.zeros((NCH * 128, 128), dtype=np.float32)
        wq_t[:EMBED, :HD] = (w_qkv[sl_q] * C_SCORE).T
        wq_t[EMBED, :HD] = b_qkv[sl_q] * C_SCORE
        wq_t[EMBED, HD] = 1.0

        wk_t = np.zeros((NCH * 128, 128), dtype=np.float32)
        wk_t[:EMBED, :HD] = w_qkv[sl_k].T
        wk_t[EMBED, :HD] = b_qkv[sl_k]
        wk_t[EMBED, HD] = SHIFT_K
        wk_t[544:561] = wk_t[512:529]  # chunk-4 row-tile replica

        # ones column at index 0 so the softmax denominator lands on
        # PSUM partition 0 (engine partition bases must be 32-aligned)
        wv_t = np.zeros((NCH * 128, HD + 2), dtype=np.float32)
        wv_t[:EMBED, 1 : HD + 1] = w_qkv[sl_v].T
        wv_t[EMBED, 1 : HD + 1] = b_qkv[sl_v]
        wv_t[EMBED, 0] = 1.0  # ones column -> softmax denominator

        wp_t = np.zeros((128, EMBED), dtype=np.float32)
        wp_t[1 : HD + 1] = w_proj[:, sl_q].T  # row 0 = 0 kills the denom row

        in_maps.append(
            {
                "xT": xT_in,
                "wq": np.ascontiguousarray(
                    wq_t.reshape(NCH, 128, 128).transpose(1, 0, 2)
                ).astype(BF16_NP),
                "wk": np.ascontiguousarray(
                    wk_t.reshape(NCH, 128, 128).transpose(1, 0, 2)
                ).astype(BF16_NP),
                "wv": np.ascontiguousarray(
                    wv_t.reshape(NCH, 128, HD + 2).transpose(1, 0, 2)
                ).astype(BF16_NP),
                "wp": wp_t.astype(BF16_NP),
                "cs": cs,
            }
        )
    return in_maps


_NC_CACHE = {}


def _get_nc(nt=NT):
    if nt not in _NC_CACHE:
        _NC_CACHE[nt] = _build_nc(nt)
    return _NC_CACHE[nt]


def kernel(x, w_qkv, b_qkv, w_proj, b_proj, _trace=False):
    from concourse.bass_utils import run_bass_kernel_spmd

    x = np.asarray(x, dtype=np.float32)
    b_proj = np.asarray(b_proj, dtype=np.float32)
    B, D, H, W, C = x.shape
    nt = D * H * W

    nc = _get_nc(nt)
    in_maps = _prep_inputs(x, w_qkv, b_qkv, w_proj, nt)
    res = run_bass_kernel_spmd(
        nc, in_maps, core_ids=list(range(NHEADS)), trace=_trace
    )
    outT = np.zeros((EMBED, nt), dtype=np.float32)
    for r in res.results:
        outT += r["y"].astype(np.float32)
    out = outT.T + b_proj
    kernel.last_results = res
    return np.ascontiguousarray(out).reshape(B, D, H, W, C)


# revision 38
# speedup vs baseline: 1.1661x; 1.1661x over previous
"""Trainium2 Bass kernel for 3D multi-head attention (nn_Attention3D).

Problem: x [1, 16, 16, 16, 528] -> full attention over N=4096 tokens,
8 heads of dim 66, qkv + out projections.

Sharding: one head per NeuronCore (8 cores). Each core computes its
head's q/k/v projections, full 4096x4096 attention, and its partial
contribution to the output projection. Host sums the 8 partials and
adds the output bias.

Key layout decisions (all host-side prep, free):
  - x is pre-transposed on host to xT [640, 4096] (C on partitions),
    with row 528 = 1.0 (bias row) and rows 529-639 = 0 padding; qkv
    biases fold into the weight matmuls via the ones row.
  - Scores are computed transposed ([k-tokens, q-tokens]) so softmax's
    sum runs over the partition dim via a ones-column in the v weights
    (the attention-value matmul accumulates the denominator for free),
    and no transposes are ever needed.
  - Scores are produced directly in "exp2-bits" units: hd^-0.5 * log2e
    * 2^23 is folded into wq/bq, and a global -0.5*2^23 shift rides on
    the spare qT/kT row 66 (a constant shift of all scores cancels in
    softmax normalization).  This lets the exp over the 16.7M-score
    matrix be SPLIT across two engines:
      * ScalarE: native Exp activation with scale=ln2*2^-23,
        bias=+0.5*ln2 in the free affine -> bf16 E.
      * VectorE: a one-instruction custom DVE op that assembles the
        fp32 BIT PATTERN of 2^t arithmetically (magic-add floor range
        reduction, quadratic mantissa polynomial, +(127+a0)*2^23) and
        writes through an int32-convert output.  The attention-value
        matmul then reads the high 2 bytes of each fp32 via a stride-2
        bf16 bitcast AP (truncation bias cancels in the softmax
        normalization since the denominator sees the same values).
    Baseline had exp 100% on ScalarE (140us, co-critical with the PE);
    the split makes phase B purely PE-bound.
  - All matmuls are bf16 (1 cyc/row + fast weight load), including the
    output projection (baseline used float32r there: ~660ns/MM vs
    ~213ns bf16).  PSUM accumulation is always fp32.
  - Phase B runs a cross-block software pipeline: the AV matmuls trail
    the score/exp stream by 3 two-chunk groups (even across q-block
    boundaries), 3x2 score PSUM banks + 2 oT-accumulator banks.  The
    last two groups of each block exp entirely on ScalarE so VectorE
    is free for the block-end reciprocal + normalize.
  - The softmax normalization is folded into the oT copy as a
    column-wise tensor_tensor multiply against the gpsimd
    partition-broadcast reciprocal, so the output projection needs no
    per-token scale.  That lets phase C emit y TRANSPOSED
    ([embed, token]): one weight load per 128-wide embed chunk, N=512
    matmuls, and one ~0.5MB DMA per embed-chunk half with
    8KB-contiguous DRAM runs (token-major y was stuck at ~65GB/s per
    queue on 1KB runs).  y partials are bf16, summed in fp32 on host.

  - Phase A packs the 17-live-row chunk-4 of the q/k projections into
    two concurrent K=32 row-tiles (host replicates the rows at
    partition offset 32), and the per-block softmax normalize is
    DEFERRED into the middle of the next block so the reciprocal +
    column-multiply never sit in VectorE's queue ahead of the exps the
    PE is waiting on at a block boundary.

Measured: 251us (baseline) -> ~177us/core at full clock (~210us if the
chip is in the P0 2.0GHz power state), rel err ~2.3e-3.  Phase B (the
4096x4096 attention itself) runs at 113us with <1.5us of PE idle --
the bf16 1-column/cycle streaming floor for scores+AV.
"""

import numpy as np

import ml_dtypes

BF16_NP = ml_dtypes.bfloat16

EMBED = 528
HD = 66
NHEADS = 8
NT = 4096
NCH = 5  # contraction chunks of 128 (640 = 528 + bias row + pad)

# exp2-bits constants (see exp_dve.py experiment)
MAGIC = 1.5 * 2.0**46
A0 = 0.414839277933763
A1 = 0.9948016962806719
A2 = 0.33717699739561857
BIAS_E = (127.0 + A0) * 2.0**23
LN2 = float(np.log(2.0))
LN2_SC = LN2 / 2.0**23
C_SCORE = float(HD**-0.5 / LN2 * 2.0**23)  # fold into wq/bq
SHIFT_K = -0.5 * 2.0**23  # qT/kT row 66 rank-1 constant shift


def _register_exp2():
    """Register the one-instruction DVE 2^x-bits op (idempotent)."""
    import concourse.dve_ops as dve_ops

    for op in dve_ops.OPS:
        if op.name == "EXP2_BITS_ANT":
            return op

    from concourse.dve_spec import (
        Spec,
        Src0,
        C0,
        C1,
        C2,
        C3,
        lower,
        _spill_c3_to_src1,
    )
    from concourse.dve_uop import DveOpSpec

    # Input is ts23 = (t - 0.5)*2^23; out_bits = fp32 bits of 2^t:
    #   p   = ts23 + 1.5*2^46  (rounds ts23 to nearest multiple of 2^23)
    #   i23 = p - 1.5*2^46     (= floor(t)*2^23 exact)
    #   X   = ts23 - i23       (= (f-0.5)*2^23, f = t-floor(t), exact)
    #   m   = X*(a1 + (a2/2^23)*X)
    #   y   = m + i23 + (127 + a0)*2^23
    # with a0 + a1 x + a2 x^2 ~= sqrt2*2^x - 1 = 2^f - 1 >= 0 on x in
    # [-0.5, 0.5]; int32-convert writeback turns the value into bits.
    p = Src0 + C0
    i23 = p - C0
    x = Src0 - i23
    m = (x * C3 + C1) * x
    y = (m + i23) + C2

    def _ref(in0, in1, c0, c1, c2):
        f32 = np.float32
        t = in0.astype(f32)
        pp = (t + f32(c0)).astype(f32)
        ii = (pp - f32(c0)).astype(f32)
        xx = (t - ii).astype(f32)
        mm = (xx * ((in1 * xx).astype(f32) + f32(c1))).astype(f32)
        return (mm + ii + f32(c2)).astype(f32)

    spec = Spec(body=_spill_c3_to_src1(y), reference=_ref)
    shas = {}
    for ver in ("v3", "v4"):
        try:
            s = DveOpSpec(
                name="EXP2_BITS_ANT", opcode=None, uops=lower(spec, ver=ver),
                rd1_en=True,
            )
            shas[ver] = s.sha(ver)
        except Exception:
            pass
    op = dve_ops.DveOp("EXP2_BITS_ANT", spec, subdim=False, uops_sha=shas)
    dve_ops.OPS.append(op)
    dve_ops._SUB_OPCODE_FOR_NAME[op.name] = (
        dve_ops._CUSTOM_DVE_ROW_BASE + len(dve_ops.OPS) - 1
    )
    dve_ops.CUSTOM_DVE_SPECS[op.name] = op.spec
    return op


def _build_nc(nt=NT):
    import concourse.tile as tile
    from concourse import bacc, mybir

    exp2_op = _register_exp2()

    F32 = mybir.dt.float32
    I32 = mybir.dt.int32
    BF16 = mybir.dt.bfloat16
    AF = mybir.ActivationFunctionType

    nkc = nt // 128  # k-token chunks
    nqb = nt // 512  # q-token blocks

    nc = bacc.Bacc("TRN2", target_bir_lowering=False, debug=False)
    xT_d = nc.dram_tensor("xT", [128, NCH, nt], BF16, kind="ExternalInput").ap()
    wq_d = nc.dram_tensor("wq", [128, NCH, 128], BF16, kind="ExternalInput").ap()
    wk_d = nc.dram_tensor("wk", [128, NCH, 128], BF16, kind="ExternalInput").ap()
    wv_d = nc.dram_tensor("wv", [128, NCH, HD + 2], BF16, kind="ExternalInput").ap()
    wp_d = nc.dram_tensor("wp", [128, EMBED], BF16, kind="ExternalInput").ap()
    cs_d = nc.dram_tensor("cs", [128, 2], F32, kind="ExternalInput").ap()
    y_d = nc.dram_tensor("y", [EMBED, nt], BF16, kind="ExternalOutput").ap()

    with tile.TileContext(nc) as tc:
        with (
            tc.tile_pool(name="const", bufs=1) as constp,
            tc.tile_pool(name="persist", bufs=1) as pp,
        ):
            wq = constp.tile([128, NCH, 128], BF16, name="wq_sb")
            wk = constp.tile([128, NCH, 128], BF16, name="wk_sb")
            wv = constp.tile([128, NCH, HD + 2], BF16, name="wv_sb")
            wp = constp.tile([128, EMBED], BF16, name="wp_sb")
            cs = constp.tile([128, 2], F32, name="cs_sb")
            warm = constp.tile([128, 1], F32, name="warm_sb")
            nc.sync.dma_start(cs[:], cs_d[:])
            c3_ap = cs[:, 0:1]
            cb_ap = cs[:, 1:2]

            # preload the exp table set early (2.7us, overlaps phase A)
            nc.scalar.activation(warm[:], cs[:, 0:1], AF.Exp, scale=0.0)

            qT = pp.tile([128, nt], BF16, name="qT")
            kT = pp.tile([128, nt], BF16, name="kT")
            oT = pp.tile([128, nt], BF16, name="oT")
            vaug = pp.tile([128, nkc, HD + 2], BF16, name="vaug")
            # junk rows 68-127 of oT feed the projection lhsT; NaN*0 = NaN.
            # (partition bases must be 32-aligned, so clear all of oT; the
            # live rows 0-67 are overwritten per q-block in phase B)
            nc.gpsimd.memset(oT[:, :], 0.0)
            # first partition_broadcast pays a ~7us gpsimd ucode load;
            # warm it up here where gpsimd is idle
            nc.gpsimd.partition_broadcast(warm[:], cs[0:1, 0:1])

            # ---------------- Phase A: qkv projections ----------------
            with (
                tc.tile_pool(name="xp", bufs=1) as xp,
                tc.tile_pool(name="psA", bufs=4, space="PSUM") as psA,
            ):
                xT = xp.tile([128, NCH, nt], BF16, name="xT_sb")
                # two HW DMA queues (sync + scalar), ~600ns trigger cost
                # each: block 0 lands chunk-by-chunk on sync so the first
                # matmul starts ~9us in; everything else is block-sized.
                b0 = slice(0, 512)
                for c in range(NCH):
                    nc.sync.dma_start(xT[:, c, b0], xT_d[:, c, b0])
                nc.scalar.dma_start(wq[:], wq_d[:])
                nc.scalar.dma_start(wk[:], wk_d[:])
                for b in range(1, nqb):
                    qs = slice(b * 512, (b + 1) * 512)
                    eng = nc.scalar if b % 2 else nc.sync
                    eng.dma_start(xT[:, :, qs], xT_d[:, :, qs])
                nc.scalar.dma_start(wv[:], wv_d[:])
                nc.sync.dma_start(wp[:], wp_d[:])

                # interleave the q and k accumulation chains (independent
                # PSUM banks) so consecutive PE matmuls pipeline
                # q/k/v interleaved: the v matmuls stream only 68
                # columns, so standalone they are weight-load bound
                # (~82ns); woven between the 213ns q/k streams their
                # LDWEIGHTS hide completely (PE pulls weight loads ahead
                # into the background buffer).
                for b in range(nqb):
                    qs = slice(b * 512, (b + 1) * 512)
                    ps_q = psA.tile([128, 512], F32, tag="qk", name="ps_q")
                    ps_k = psA.tile([128, 512], F32, tag="qk", name="ps_k")
                    psvs = [
                        psA.tile([128, HD + 2], F32, tag="v", name="ps_v")
                        for _ in range(4)
                    ]
                    for c in range(NCH):
                        if c < NCH - 1:
                            for w, ps in ((wq, ps_q), (wk, ps_k)):
                                nc.tensor.matmul(
                                    ps[:],
                                    w[:, c, :],
                                    xT[:, c, qs],
                                    start=(c == 0),
                                    stop=False,
                                    skip_group_check=True,
                                )
                        else:
                            # chunk 4 has only 17 live rows; q on rows
                            # 0-31, k on rows 32-63 (host-replicated),
                            # streaming concurrently
                            nc.tensor.matmul(
                                ps_q[:],
                                wq[0:32, NCH - 1, :],
                                xT[0:32, NCH - 1, qs],
                                start=False,
                                stop=True,
                                skip_group_check=True,
                            )
                            nc.tensor.matmul(
                                ps_k[:],
                                wk[32:64, NCH - 1, :],
                                xT[32:64, NCH - 1, qs],
                                start=False,
                                stop=True,
                                skip_group_check=True,
                                tile_position=(32, 0),
                            )
                        for i in range(4):
                            t = 4 * b + i
                            ts_ = slice(t * 128, (t + 1) * 128)
                            nc.tensor.matmul(
                                psvs[i][:],
                                xT[:, c, ts_],
                                wv[:, c, :],
                                start=(c == 0),
                                stop=(c == NCH - 1),
                                skip_group_check=True,
                            )
                    nc.vector.tensor_copy(qT[:, qs], ps_q[:])
                    nc.vector.tensor_copy(kT[:, qs], ps_k[:])
                    for i in range(4):
                        nc.scalar.activation(
                            vaug[:, 4 * b + i, :], psvs[i][:], AF.Copy
                        )

            # ---------------- Phase B: attention ----------------
            # 2-chunk score groups, triple-buffered (3x2 PSUM banks) + 2
            # banks for the oT accumulator = 8.  Per group, chunk 0's exp
            # runs on ScalarE (bf16 E) and chunk 1's on VectorE (exp2-bits
            # int32) -- different PSUM banks, so the reads are parallel.
            # AV(g) is emitted after scores(g+2), giving the exps ~1.3us
            # of PE-time cover.  The softmax normalization is folded into
            # the oT copy (column-wise multiply against the partition-
            # broadcast reciprocal), so phase C needs no per-token scale
            # and can emit y transposed with large DMA runs.
            with (
                tc.tile_pool(name="eps", bufs=4) as eps,
                tc.tile_pool(name="edp", bufs=4) as edp,
                tc.tile_pool(name="rp", bufs=2) as rp,
                tc.tile_pool(name="rbp", bufs=2) as rbp,
                tc.tile_pool(name="psS", bufs=3, space="PSUM") as psS,
                tc.tile_pool(name="psO", bufs=2, space="PSUM") as psO,
            ):
                def emit_av(b, o_ps, g0, Es, Ed):
                    qs = slice(b * 512, (b + 1) * 512)
                    for j in range(2):
                        kc = g0 + j
                        if Ed is None:
                            rhs = Es[:, j, :]
                        elif j == 0:
                            rhs = Es[:]
                        else:
                            rhs = (
                                Ed[:]
                                .bitcast(BF16)
                                .rearrange("p (n two) -> p n two", two=2)[:, :, 1]
                            )
                        nc.tensor.matmul(
                            o_ps[:],
                            vaug[:, kc, :],
                            rhs,
                            start=(kc == 0),
                            stop=(kc == nkc - 1),
                            skip_group_check=True,
                        )
                    if g0 + 2 == nkc:
                        # block finished: defer the normalize a few groups
                        # so the recip+mul don't sit in VectorE's queue
                        # ahead of the next block's exps at the boundary
                        norm_q.append((b, o_ps))

                def emit_norm():
                    nb, no_ps = norm_q.pop(0)
                    nqs = slice(nb * 512, (nb + 1) * 512)
                    recip = rp.tile([1, 512], F32, name="recip")
                    nc.vector.reciprocal_approx_fast(recip[:], no_ps[0:1, :])
                    recipB = rbp.tile([128, 512], F32, name="recipB")
                    nc.gpsimd.partition_broadcast(recipB[:], recip[:])
                    nc.vector.tensor_mul(
                        oT[: HD + 2, nqs], no_ps[:], recipB[: HD + 2, :]
                    )

                # the AV pipeline trails the score/exp stream by 3 groups
                # ACROSS block boundaries, so the last exps of a block get
                # PE cover from the next block's first scores
                pend = []
                norm_q = []
                for b in range(nqb):
                    qs = slice(b * 512, (b + 1) * 512)
                    o_ps = psO.tile([HD + 2, 512], F32, name="o_ps")
                    for g in range(nkc // 2):
                        g0 = 2 * g
                        sc = psS.tile([128, 2, 512], F32, tag="sc", name="sc")
                        for j in range(2):
                            kc = g0 + j
                            nc.tensor.matmul(
                                sc[:, j, :],
                                kT[:, kc * 128 : (kc + 1) * 128],
                                qT[:, qs],
                                start=True,
                                stop=True,
                            )
                        if g >= nkc // 2 - 2:
                            # last two groups: both chunks on ScalarE (one
                            # wide activation) so VectorE is free for the
                            # block-end recip + normalize without stalling
                            # the next block's first AVs
                            Es = eps.tile(
                                [128, 2, 512], BF16, tag="Es2", name="Es2"
                            )
                            nc.scalar.activation(
                                Es[:],
                                sc[:],
                                AF.Exp,
                                scale=LN2_SC,
                                bias=cb_ap,
                            )
                            Ed = None
                        else:
                            Es = eps.tile([128, 512], BF16, tag="Es", name="Es")
                            Ed = edp.tile([128, 512], I32, tag="Ed", name="Ed")
                            nc.scalar.activation(
                                Es[:], sc[:, 0, :], AF.Exp, scale=LN2_SC, bias=cb_ap
                            )
                            nc.vector._custom_dve(
                                exp2_op,
                                out=Ed[:],
                                in0=sc[:, 1, :],
                                in1=c3_ap,
                                s0=MAGIC,
                                s1=A1,
                                imm2=BIAS_E,
                            )
                        pend.append((b, o_ps, g0, Es, Ed))
                        if len(pend) > 3:
                            emit_av(*pend.pop(0))
                        if norm_q and g == 4:
                            emit_norm()
                for p_ in pend:
                    emit_av(*p_)
                while norm_q:
                    emit_norm()

            # ---------------- Phase C: output projection ----------------
            # y is produced TRANSPOSED ([embed, token]): one weight load
            # per 128-wide embed chunk, N=512 matmuls over token blocks,
            # single-bank PSUM tiles, and one ~1MB DMA per embed chunk
            # with 8KB-contiguous runs (token-major y needed 32 DMAs of
            # 1056B runs at ~65GB/s/queue, which dominated phase C).
            with (
                tc.tile_pool(name="yp", bufs=2) as yp,
                tc.tile_pool(name="psY", bufs=4, space="PSUM") as psY,
            ):
                for e in range(5):
                    ew = 128 if e < 4 else EMBED - 512  # last chunk: 16
                    es_ = slice(e * 128, e * 128 + ew)
                    ysb = yp.tile([128, nt], BF16, tag="ysb", name="ysb")
                    for th in range(nqb // 2):
                        yps = psY.tile([128, 2, 512], F32, name="yps")
                        for i in range(2):
                            tb = 2 * th + i
                            ts_ = slice(tb * 512, (tb + 1) * 512)
                            nc.tensor.matmul(
                                yps[:ew, i, :],
                                wp[:, es_],
                                oT[:, ts_],
                                start=True,
                                stop=True,
                            )
                        hs = slice(2 * th * 512, (2 * th + 2) * 512)
                        if th % 2 == 0:
                            nc.vector.tensor_copy(ysb[:ew, hs], yps[:ew, :, :])
                        else:
                            nc.scalar.activation(
                                ysb[:ew, hs], yps[:ew, :, :], AF.Copy
                            )
                    h = nt // 2
                    engs = (nc.sync, nc.scalar) if e % 2 == 0 else (nc.scalar, nc.sync)
                    engs[0].dma_start(y_d[es_, :h], ysb[:ew, :h])
                    engs[1].dma_start(y_d[es_, h:], ysb[:ew, h:])

    nc.compile()
    return nc


def _prep_inputs(x, w_qkv, b_qkv, w_proj, nt):
    """Host-side shard prep: returns list of 8 in_maps."""
    x = np.asarray(x, dtype=np.float32)
    w_qkv = np.asarray(w_qkv, dtype=np.float32)
    b_qkv = np.asarray(b_qkv, dtype=np.float32)
    w_proj = np.asarray(w_proj, dtype=np.float32)

    xt = x.reshape(nt, EMBED)
    xT_pad = np.zeros((NCH * 128, nt), dtype=np.float32)
    xT_pad[:EMBED] = xt.T
    xT_pad[EMBED] = 1.0
    # replica of chunk-4's 17 live rows at partition offset 32 for the
    # k-projection row-tile (rows 544-560 = rows 512-528)
    xT_pad[544:561] = xT_pad[512:529]
    # [128, NCH, nt]: partition-major so one DMA covers a token block
    xT_in = np.ascontiguousarray(
        xT_pad.reshape(NCH, 128, nt).transpose(1, 0, 2)
    ).astype(BF16_NP)

    cs = np.zeros((128, 2), dtype=np.float32)
    cs[:, 0] = A2 / 2.0**23
    cs[:, 1] = 0.5 * LN2

    in_maps = []
    for h in range(NHEADS):
        sl_q = slice(h * HD, (h + 1) * HD)
        sl_k = slice(EMBED + h * HD, EMBED + (h + 1) * HD)
        sl_v = slice(2 * EMBED + h * HD, 2 * EMBED + (h + 1) * HD)

        # q side carries the hd^-0.5 * log2e * 2^23 score scaling; the
        # spare column 66 carries the global -0.5*2^23 shift (qT66 = 1,
        # kT66 = -2^22) that the exp2-bits op's range reduction needs.
        wq_t = np.zeros((NCH * 128, 128), dtype=np.float32)
        wq_t[:EMBED, :HD] = (w_qkv[sl_q] * C_SCORE).T
        wq_t[EMBED, :HD] = b_qkv[sl_q] * C_SCORE
        wq_t[EMBED, HD] = 1.0

        wk_t = np.zeros((NCH * 128, 128), dtype=np.float32)
        wk_t[:EMBED, :HD] = w_qkv[sl_k].T
        wk_t[EMBED, :HD] = b_qkv[sl_k]
        wk_t[EMBED, HD] = SHIFT_K
        wk_t[544:561] = wk_t[512:529]  # chunk-4 row-tile replica

        # ones column at index 0 so the softmax denominator lands on
        # PSUM partition 0 (engine partition bases must be 32-aligned)
        wv_t = np.zeros((NCH * 128, HD + 2), dtype=np.float32)
        wv_t[:EMBED, 1 : HD + 1] = w_qkv[sl_v].T
        wv_t[EMBED, 1 : HD + 1] = b_qkv[sl_v]
        wv_t[EMBED, 0] = 1.0  # ones column -> softmax denominator

        wp_t = np.zeros((128, EMBED), dtype=np.float32)
        wp_t[1 : HD + 1] = w_proj[:, sl_q].T  # row 0 = 0 kills the denom row

        in_maps.append(
            {
                "xT": xT_in,
                "wq": np.ascontiguousarray(
                    wq_t.reshape(NCH, 128, 128).transpose(1, 0, 2)
                ).astype(BF16_NP),
                "wk": np.ascontiguousarray(
                    wk_t.reshape(NCH, 128, 128).transpose(1, 0, 2)
                ).astype(BF16_NP),
                "wv": np.ascontiguousarray(
                    wv_t.reshape(NCH, 128, HD + 2).transpose(1, 0, 2)
                ).astype(BF16_NP),
                "wp": wp_t.astype(BF16_NP),
                "cs": cs,
            }
        )
    return in_maps


_NC_CACHE = {}


def _get_nc(nt=NT):
    if nt not in _NC_CACHE:
        _NC_CACHE[nt] = _build_nc(nt)
    return _NC_CACHE[nt]


def kernel(x, w_qkv, b_qkv, w_proj, b_proj, _trace=False):
    from concourse.bass_utils import run_bass_kernel_spmd

    x = np.asarray(x, dtype=np.float32)
    b_proj = np.asarray(b_proj, dtype=np.float32)
    B, D, H, W, C = x.shape
    nt = D * H * W

    nc = _get_nc(nt)
    in_maps = _prep_inputs(x, w_qkv, b_qkv, w_proj, nt)
    res = run_bass_kernel_spmd(
        nc, in_maps, core_ids=list(range(NHEADS)), trace=_trace
    )
    outT = np.zeros((EMBED, nt), dtype=np.float32)
    for r in res.results:
        outT += r["y"].astype(np.float32)
    out = outT.T + b_proj
    kernel.last_results = res
    return np.ascontiguousarray(out).reshape(B, D, H, W, C)


# revision 39
# speedup vs baseline: 1.1768x; 1.0092x over previous
"""Trainium2 Bass kernel for 3D multi-head attention (nn_Attention3D).

Problem: x [1, 16, 16, 16, 528] -> full attention over N=4096 tokens,
8 heads of dim 66, qkv + out projections.

Sharding: one head per NeuronCore (8 cores). Each core computes its
head's q/k/v projections, full 4096x4096 attention, and its partial
contribution to the output projection. Host sums the 8 partials and
adds the output bias.

Key layout decisions (all host-side prep, free):
  - x is pre-transposed on host to xT [640, 4096] (C on partitions),
    with row 528 = 1.0 (bias row) and rows 529-639 = 0 padding; qkv
    biases fold into the weight matmuls via the ones row.
  - Scores are computed transposed ([k-tokens, q-tokens]) so softmax's
    sum runs over the partition dim via a ones-column in the v weights
    (the attention-value matmul accumulates the denominator for free),
    and no transposes are ever needed.
  - Scores are produced directly in "exp2-bits" units: hd^-0.5 * log2e
    * 2^23 is folded into wq/bq, and a global -0.5*2^23 shift rides on
    the spare qT/kT row 66 (a constant shift of all scores cancels in
    softmax normalization).  This lets the exp over the 16.7M-score
    matrix be SPLIT across two engines:
      * ScalarE: native Exp activation with scale=ln2*2^-23,
        bias=+0.5*ln2 in the free affine -> bf16 E.
      * VectorE: a one-instruction custom DVE op that assembles the
        fp32 BIT PATTERN of 2^t arithmetically (magic-add floor range
        reduction, quadratic mantissa polynomial, +(127+a0)*2^23) and
        writes through an int32-convert output.  The attention-value
        matmul then reads the high 2 bytes of each fp32 via a stride-2
        bf16 bitcast AP (truncation bias cancels in the softmax
        normalization since the denominator sees the same values).
    Baseline had exp 100% on ScalarE (140us, co-critical with the PE);
    the split makes phase B purely PE-bound.
  - All matmuls are bf16 (1 cyc/row + fast weight load), including the
    output projection (baseline used float32r there: ~660ns/MM vs
    ~213ns bf16).  PSUM accumulation is always fp32.
  - Phase B runs a cross-block software pipeline: the AV matmuls trail
    the score/exp stream by 3 two-chunk groups (even across q-block
    boundaries), 3x2 score PSUM banks + 2 oT-accumulator banks.  The
    last two groups of each block exp entirely on ScalarE so VectorE
    is free for the block-end reciprocal + normalize.
  - The softmax normalization is folded into the oT copy as a
    column-wise tensor_tensor multiply against the gpsimd
    partition-broadcast reciprocal, so the output projection needs no
    per-token scale.  That lets phase C emit y TRANSPOSED
    ([embed, token]): one weight load per 128-wide embed chunk, N=512
    matmuls, and one ~0.5MB DMA per embed-chunk half with
    8KB-contiguous DRAM runs (token-major y was stuck at ~65GB/s per
    queue on 1KB runs).  y partials are bf16, summed in fp32 on host.

  - Phase A packs the 17-live-row chunk-4 of the q/k projections into
    two concurrent K=32 row-tiles (host replicates the rows at
    partition offset 32), and the per-block softmax normalize is
    DEFERRED into the middle of the next block so the reciprocal +
    column-multiply never sit in VectorE's queue ahead of the exps the
    PE is waiting on at a block boundary.

Measured: 251us (baseline) -> ~177us/core at full clock (~210us if the
chip is in the P0 2.0GHz power state), rel err ~2.3e-3.  Phase B (the
4096x4096 attention itself) runs at 113us with <1.5us of PE idle --
the bf16 1-column/cycle streaming floor for scores+AV.
"""

import numpy as np

import ml_dtypes

BF16_NP = ml_dtypes.bfloat16

EMBED = 528
HD = 66
NHEADS = 8
NT = 4096
NCH = 5  # contraction chunks of 128 (640 = 528 + bias row + pad)

# exp2-bits constants (see exp_dve.py experiment)
MAGIC = 1.5 * 2.0**46
A0 = 0.414839277933763
A1 = 0.9948016962806719
A2 = 0.33717699739561857
BIAS_E = (127.0 + A0) * 2.0**23
LN2 = float(np.log(2.0))
LN2_SC = LN2 / 2.0**23
C_SCORE = float(HD**-0.5 / LN2 * 2.0**23)  # fold into wq/bq
SHIFT_K = -0.5 * 2.0**23  # qT/kT row 66 rank-1 constant shift


def _register_exp2():
    """Register the one-instruction DVE 2^x-bits op (idempotent)."""
    import concourse.dve_ops as dve_ops

    for op in dve_ops.OPS:
        if op.name == "EXP2_BITS_ANT":
            return op

    from concourse.dve_spec import (
        Spec,
        Src0,
        C0,
        C1,
        C2,
        C3,
        lower,
        _spill_c3_to_src1,
    )
    from concourse.dve_uop import DveOpSpec

    # Input is ts23 = (t - 0.5)*2^23; out_bits = fp32 bits of 2^t:
    #   p   = ts23 + 1.5*2^46  (rounds ts23 to nearest multiple of 2^23)
    #   i23 = p - 1.5*2^46     (= floor(t)*2^23 exact)
    #   X   = ts23 - i23       (= (f-0.5)*2^23, f = t-floor(t), exact)
    #   m   = X*(a1 + (a2/2^23)*X)
    #   y   = m + i23 + (127 + a0)*2^23
    # with a0 + a1 x + a2 x^2 ~= sqrt2*2^x - 1 = 2^f - 1 >= 0 on x in
    # [-0.5, 0.5]; int32-convert writeback turns the value into bits.
    p = Src0 + C0
    i23 = p - C0
    x = Src0 - i23
    m = (x * C3 + C1) * x
    y = (m + i23) + C2

    def _ref(in0, in1, c0, c1, c2):
        f32 = np.float32
        t = in0.astype(f32)
        pp = (t + f32(c0)).astype(f32)
        ii = (pp - f32(c0)).astype(f32)
        xx = (t - ii).astype(f32)
        mm = (xx * ((in1 * xx).astype(f32) + f32(c1))).astype(f32)
        return (mm + ii + f32(c2)).astype(f32)

    spec = Spec(body=_spill_c3_to_src1(y), reference=_ref)
    shas = {}
    for ver in ("v3", "v4"):
        try:
            s = DveOpSpec(
                name="EXP2_BITS_ANT", opcode=None, uops=lower(spec, ver=ver),
                rd1_en=True,
            )
            shas[ver] = s.sha(ver)
        except Exception:
            pass
    op = dve_ops.DveOp("EXP2_BITS_ANT", spec, subdim=False, uops_sha=shas)
    dve_ops.OPS.append(op)
    dve_ops._SUB_OPCODE_FOR_NAME[op.name] = (
        dve_ops._CUSTOM_DVE_ROW_BASE + len(dve_ops.OPS) - 1
    )
    dve_ops.CUSTOM_DVE_SPECS[op.name] = op.spec
    return op


def _build_nc(nt=NT):
    import concourse.tile as tile
    from concourse import bacc, mybir

    exp2_op = _register_exp2()

    F32 = mybir.dt.float32
    I32 = mybir.dt.int32
    BF16 = mybir.dt.bfloat16
    AF = mybir.ActivationFunctionType

    nkc = nt // 128  # k-token chunks
    nqb = nt // 512  # q-token blocks

    nc = bacc.Bacc("TRN2", target_bir_lowering=False, debug=False)
    xT_d = nc.dram_tensor("xT", [128, NCH, nt], BF16, kind="ExternalInput").ap()
    wq_d = nc.dram_tensor("wq", [128, NCH, 128], BF16, kind="ExternalInput").ap()
    wk_d = nc.dram_tensor("wk", [128, NCH, 128], BF16, kind="ExternalInput").ap()
    wv_d = nc.dram_tensor("wv", [128, NCH, HD + 2], BF16, kind="ExternalInput").ap()
    wp_d = nc.dram_tensor("wp", [128, EMBED], BF16, kind="ExternalInput").ap()
    cs_d = nc.dram_tensor("cs", [128, 2], F32, kind="ExternalInput").ap()
    y_d = nc.dram_tensor("y", [EMBED, nt], BF16, kind="ExternalOutput").ap()

    with tile.TileContext(nc) as tc:
        with (
            tc.tile_pool(name="const", bufs=1) as constp,
            tc.tile_pool(name="persist", bufs=1) as pp,
        ):
            wq = constp.tile([128, NCH, 128], BF16, name="wq_sb")
            wk = constp.tile([128, NCH, 128], BF16, name="wk_sb")
            wv = constp.tile([128, NCH, HD + 2], BF16, name="wv_sb")
            wp = constp.tile([128, EMBED], BF16, name="wp_sb")
            cs = constp.tile([128, 2], F32, name="cs_sb")
            warm = constp.tile([128, 1], F32, name="warm_sb")
            nc.sync.dma_start(cs[:], cs_d[:])
            c3_ap = cs[:, 0:1]
            cb_ap = cs[:, 1:2]

            # preload the exp table set early (2.7us, overlaps phase A)
            nc.scalar.activation(warm[:], cs[:, 0:1], AF.Exp, scale=0.0)

            qT = pp.tile([128, nt], BF16, name="qT")
            kT = pp.tile([128, nt], BF16, name="kT")
            oT = pp.tile([128, nt], BF16, name="oT")
            vaug = pp.tile([128, nkc, HD + 2], BF16, name="vaug")
            # junk rows 68-127 of oT feed the projection lhsT; NaN*0 = NaN.
            # (partition bases must be 32-aligned, so clear all of oT; the
            # live rows 0-67 are overwritten per q-block in phase B)
            nc.gpsimd.memset(oT[:, :], 0.0)
            # first partition_broadcast pays a ~7us gpsimd ucode load;
            # warm it up here where gpsimd is idle
            nc.gpsimd.partition_broadcast(warm[:], cs[0:1, 0:1])

            # ---------------- Phase A: qkv projections ----------------
            with (
                tc.tile_pool(name="xp", bufs=1) as xp,
                tc.tile_pool(name="psA", bufs=4, space="PSUM") as psA,
            ):
                xT = xp.tile([128, NCH, nt], BF16, name="xT_sb")
                # two HW DMA queues (sync + scalar), ~600ns trigger cost
                # each: block 0 lands chunk-by-chunk on sync so the first
                # matmul starts ~9us in; everything else is block-sized.
                b0 = slice(0, 512)
                for c in range(NCH):
                    nc.sync.dma_start(xT[:, c, b0], xT_d[:, c, b0])
                nc.scalar.dma_start(wq[:], wq_d[:])
                nc.scalar.dma_start(wk[:], wk_d[:])
                for b in range(1, nqb):
                    qs = slice(b * 512, (b + 1) * 512)
                    eng = nc.scalar if b % 2 else nc.sync
                    eng.dma_start(xT[:, :, qs], xT_d[:, :, qs])
                nc.scalar.dma_start(wv[:], wv_d[:])
                nc.sync.dma_start(wp[:], wp_d[:])

                # interleave the q and k accumulation chains (independent
                # PSUM banks) so consecutive PE matmuls pipeline
                for b in range(nqb):
                    qs = slice(b * 512, (b + 1) * 512)
                    ps_q = psA.tile([128, 512], F32, tag="qk", name="ps_q")
                    ps_k = psA.tile([128, 512], F32, tag="qk", name="ps_k")
                    for c in range(NCH - 1):
                        for w, ps in ((wq, ps_q), (wk, ps_k)):
                            nc.tensor.matmul(
                                ps[:],
                                w[:, c, :],
                                xT[:, c, qs],
                                start=(c == 0),
                                stop=False,
                                skip_group_check=True,
                            )
                    # chunk 4 has only 17 live rows (features 512-527 +
                    # bias row); run q's as a K=32 row-tile on rows 0-31
                    # and k's on rows 32-63 (host replicates the rows
                    # there) so the two matmuls stream concurrently
                    nc.tensor.matmul(
                        ps_q[:],
                        wq[0:32, NCH - 1, :],
                        xT[0:32, NCH - 1, qs],
                        start=False,
                        stop=True,
                        skip_group_check=True,
                    )
                    nc.tensor.matmul(
                        ps_k[:],
                        wk[32:64, NCH - 1, :],
                        xT[32:64, NCH - 1, qs],
                        start=False,
                        stop=True,
                        skip_group_check=True,
                        tile_position=(32, 0),
                    )
                    nc.vector.tensor_copy(qT[:, qs], ps_q[:])
                    nc.vector.tensor_copy(kT[:, qs], ps_k[:])
                # v: two token-block chains in flight; copies on ScalarE
                # (VectorE owns the qT/kT casts, ScalarE is idle here)
                for t0 in range(0, nkc, 2):
                    psvs = [
                        psA.tile([128, HD + 2], F32, tag="v", name="ps_v")
                        for _ in range(2)
                    ]
                    for c in range(NCH):
                        for i in range(2):
                            ts_ = slice((t0 + i) * 128, (t0 + i + 1) * 128)
                            nc.tensor.matmul(
                                psvs[i][:],
                                xT[:, c, ts_],
                                wv[:, c, :],
                                start=(c == 0),
                                stop=(c == NCH - 1),
                            )
                    for i in range(2):
                        nc.scalar.activation(
                            vaug[:, t0 + i, :], psvs[i][:], AF.Copy
                        )

            # ---------------- Phase B: attention ----------------
            # 2-chunk score groups, triple-buffered (3x2 PSUM banks) + 2
            # banks for the oT accumulator = 8.  Per group, chunk 0's exp
            # runs on ScalarE (bf16 E) and chunk 1's on VectorE (exp2-bits
            # int32) -- different PSUM banks, so the reads are parallel.
            # AV(g) is emitted after scores(g+2), giving the exps ~1.3us
            # of PE-time cover.  The softmax normalization is folded into
            # the oT copy (column-wise multiply against the partition-
            # broadcast reciprocal), so phase C needs no per-token scale
            # and can emit y transposed with large DMA runs.
            with (
                tc.tile_pool(name="eps", bufs=4) as eps,
                tc.tile_pool(name="edp", bufs=4) as edp,
                tc.tile_pool(name="rp", bufs=2) as rp,
                tc.tile_pool(name="rbp", bufs=2) as rbp,
                tc.tile_pool(name="psS", bufs=3, space="PSUM") as psS,
                tc.tile_pool(name="psO", bufs=2, space="PSUM") as psO,
            ):
                def emit_av(b, o_ps, g0, Es, Ed):
                    qs = slice(b * 512, (b + 1) * 512)
                    for j in range(2):
                        kc = g0 + j
                        if Ed is None:
                            rhs = Es[:, j, :]
                        elif j == 0:
                            rhs = Es[:]
                        else:
                            rhs = (
                                Ed[:]
                                .bitcast(BF16)
                                .rearrange("p (n two) -> p n two", two=2)[:, :, 1]
                            )
                        nc.tensor.matmul(
                            o_ps[:],
                            vaug[:, kc, :],
                            rhs,
                            start=(kc == 0),
                            stop=(kc == nkc - 1),
                            skip_group_check=True,
                        )
                    if g0 + 2 == nkc:
                        # block finished: defer the normalize a few groups
                        # so the recip+mul don't sit in VectorE's queue
                        # ahead of the next block's exps at the boundary
                        norm_q.append((b, o_ps))

                def emit_norm():
                    nb, no_ps = norm_q.pop(0)
                    nqs = slice(nb * 512, (nb + 1) * 512)
                    recip = rp.tile([1, 512], F32, name="recip")
                    nc.vector.reciprocal_approx_fast(recip[:], no_ps[0:1, :])
                    recipB = rbp.tile([128, 512], F32, name="recipB")
                    nc.gpsimd.partition_broadcast(recipB[:], recip[:])
                    nc.vector.tensor_mul(
                        oT[: HD + 2, nqs], no_ps[:], recipB[: HD + 2, :]
                    )

                # the AV pipeline trails the score/exp stream by 3 groups
                # ACROSS block boundaries, so the last exps of a block get
                # PE cover from the next block's first scores
                pend = []
                norm_q = []
                for b in range(nqb):
                    qs = slice(b * 512, (b + 1) * 512)
                    o_ps = psO.tile([HD + 2, 512], F32, name="o_ps")
                    for g in range(nkc // 2):
                        g0 = 2 * g
                        sc = psS.tile([128, 2, 512], F32, tag="sc", name="sc")
                        for j in range(2):
                            kc = g0 + j
                            nc.tensor.matmul(
                                sc[:, j, :],
                                kT[:, kc * 128 : (kc + 1) * 128],
                                qT[:, qs],
                                start=True,
                                stop=True,
                            )
                        if g >= nkc // 2 - 2:
                            # last two groups: both chunks on ScalarE (one
                            # wide activation) so VectorE is free for the
                            # block-end recip + normalize without stalling
                            # the next block's first AVs
                            Es = eps.tile(
                                [128, 2, 512], BF16, tag="Es2", name="Es2"
                            )
                            nc.scalar.activation(
                                Es[:],
                                sc[:],
                                AF.Exp,
                                scale=LN2_SC,
                                bias=cb_ap,
                            )
                            Ed = None
                        else:
                            Es = eps.tile([128, 512], BF16, tag="Es", name="Es")
                            Ed = edp.tile([128, 512], I32, tag="Ed", name="Ed")
                            nc.scalar.activation(
                                Es[:], sc[:, 0, :], AF.Exp, scale=LN2_SC, bias=cb_ap
                            )
                            nc.vector._custom_dve(
                                exp2_op,
                                out=Ed[:],
                                in0=sc[:, 1, :],
                                in1=c3_ap,
                                s0=MAGIC,
                                s1=A1,
                                imm2=BIAS_E,
                            )
                        pend.append((b, o_ps, g0, Es, Ed))
                        if len(pend) > 3:
                            emit_av(*pend.pop(0))
                        if norm_q and g == 4:
                            emit_norm()
                for p_ in pend:
                    emit_av(*p_)
                while norm_q:
                    emit_norm()

            # ---------------- Phase C: output projection ----------------
            # y is produced TRANSPOSED ([embed, token]): one weight load
            # per 128-wide embed chunk, N=512 matmuls over token blocks,
            # single-bank PSUM tiles, and one ~1MB DMA per embed chunk
            # with 8KB-contiguous runs (token-major y needed 32 DMAs of
            # 1056B runs at ~65GB/s/queue, which dominated phase C).
            with (
                tc.tile_pool(name="yp", bufs=2) as yp,
                tc.tile_pool(name="psY", bufs=4, space="PSUM") as psY,
            ):
                for e in range(5):
                    ew = 128 if e < 4 else EMBED - 512  # last chunk: 16
                    es_ = slice(e * 128, e * 128 + ew)
                    ysb = yp.tile([128, nt], BF16, tag="ysb", name="ysb")
                    for th in range(nqb // 2):
                        yps = psY.tile([128, 2, 512], F32, name="yps")
                        for i in range(2):
                            tb = 2 * th + i
                            ts_ = slice(tb * 512, (tb + 1) * 512)
                            nc.tensor.matmul(
                                yps[:ew, i, :],
                                wp[:, es_],
                                oT[:, ts_],
                                start=True,
                                stop=True,
                            )
                        hs = slice(2 * th * 512, (2 * th + 2) * 512)
                        if th % 2 == 0:
                            nc.vector.tensor_copy(ysb[:ew, hs], yps[:ew, :, :])
                        else:
                            nc.scalar.activation(
                                ysb[:ew, hs], yps[:ew, :, :], AF.Copy
                            )
                    h = nt // 2
                    engs = (nc.sync, nc.scalar) if e % 2 == 0 else (nc.scalar, nc.sync)
                    engs[0].dma_start(y_d[es_, :h], ysb[:ew, :h])
                    engs[1].dma_start(y_d[es_, h:], ysb[:ew, h:])

    nc.compile()
    return nc


def _prep_inputs(x, w_qkv, b_qkv, w_proj, nt):
    """Host-side shard prep: returns list of 8 in_maps."""
    x = np.asarray(x, dtype=np.float32)
    w_qkv = np.asarray(w_qkv, dtype=np.float32)
    b_qkv = np.asarray(b_qkv, dtype=np.float32)
    w_proj = np.asarray(w_proj, dtype=np.float32)

    xt = x.reshape(nt, EMBED)
    xT_pad = np.zeros((NCH * 128, nt), dtype=np.float32)
    xT_pad[:EMBED] = xt.T
    xT_pad[EMBED] = 1.0
    # replica of chunk-4's 17 live rows at partition offset 32 for the
    # k-projection row-tile (rows 544-560 = rows 512-528)
    xT_pad[544:561] = xT_pad[512:529]
    # [128, NCH, nt]: partition-major so one DMA covers a token block
    xT_in = np.ascontiguousarray(
        xT_pad.reshape(NCH, 128, nt).transpose(1, 0, 2)
    ).astype(BF16_NP)

    cs = np.zeros((128, 2), dtype=np.float32)
    cs[:, 0] = A2 / 2.0**23
    cs[:, 1] = 0.5 * LN2

    in_maps = []
    for h in range(NHEADS):
        sl_q = slice(h * HD, (h + 1) * HD)
        sl_k = slice(EMBED + h * HD, EMBED + (h + 1) * HD)
        sl_v = slice(2 * EMBED + h * HD, 2 * EMBED + (h + 1) * HD)

        # q side carries the hd^-0.5 * log2e * 2^23 score scaling; the
        # spare column 66 carries the global -0.5*2^23 shift (qT66 = 1,
        # kT66 = -2^22) that the exp2-bits op's range reduction needs.
        wq_t = np.zeros((NCH * 128, 128), dtype=np.float32)
        wq_t[:EMBED, :HD] = (w_qkv[sl_q] * C_SCORE).T
        wq_t[EMBED, :HD] = b_qkv[sl_q] * C_SCORE
        wq_t[EMBED, HD] = 1.0

        wk_t = np.zeros((NCH * 128, 128), dtype=np.float32)
        wk_t[:EMBED, :HD] = w_qkv[sl_k].T
        wk_t[EMBED, :HD] = b_qkv[sl_k]
        wk_t[EMBED, HD] = SHIFT_K
        wk_t[544:561] = wk_t[512:529]  # chunk-4 row-tile replica

        # ones column at index 0 so the softmax denominator lands on
        # PSUM partition 0 (engine partition bases must be 32-aligned)
        wv_t = np.zeros((NCH * 128, HD + 2), dtype=np.float32)
        wv_t[:EMBED, 1 : HD + 1] = w_qkv[sl_v].T
        wv_t[EMBED, 1 : HD + 1] = b_qkv[sl_v]
        wv_t[EMBED, 0] = 1.0  # ones column -> softmax denominator

        wp_t = np.zeros((128, EMBED), dtype=np.float32)
        wp_t[1 : HD + 1] = w_proj[:, sl_q].T  # row 0 = 0 kills the denom row

        in_maps.append(
            {
                "xT": xT_in,
                "wq": np.ascontiguousarray(
                    wq_t.reshape(NCH, 128, 128).transpose(1, 0, 2)
                ).astype(BF16_NP),
                "wk": np.ascontiguousarray(
                    wk_t.reshape(NCH, 128, 128).transpose(1, 0, 2)
                ).astype(BF16_NP),
                "wv": np.ascontiguousarray(
                    wv_t.reshape(NCH, 128, HD + 2).transpose(1, 0, 2)
                ).astype(BF16_NP),
                "wp": wp_t.astype(BF16_NP),
                "cs": cs,
            }
        )
    return in_maps


_NC_CACHE = {}


def _get_nc(nt=NT):
    if nt not in _NC_CACHE:
        _NC_CACHE[nt] = _build_nc(nt)
    return _NC_CACHE[nt]


def kernel(x, w_qkv, b_qkv, w_proj, b_proj, _trace=False):
    from concourse.bass_utils import run_bass_kernel_spmd

    x = np.asarray(x, dtype=np.float32)
    b_proj = np.asarray(b_proj, dtype=np.float32)
    B, D, H, W, C = x.shape
    nt = D * H * W

    nc = _get_nc(nt)
    in_maps = _prep_inputs(x, w_qkv, b_qkv, w_proj, nt)
    res = run_bass_kernel_spmd(
        nc, in_maps, core_ids=list(range(NHEADS)), trace=_trace
    )
    outT = np.zeros((EMBED, nt), dtype=np.float32)
    for r in res.results:
        outT += r["y"].astype(np.float32)
    out = outT.T + b_proj
    kernel.last_results = res
    return np.ascontiguousarray(out).reshape(B, D, H, W, C)
